# revision 62
# baseline (speedup 1.0000x reference)
"""Trainium2 Bass kernel for nn_Angles2BMatrixAB.  (~72.8us median,
rel err 4.4e-3; from a 106.5us baseline.)

Math: the reference's F^q_i = M_{i-1} dB_i/dq M_i^{-1} collapses to the
geometric Jacobian of a revolute chain:
    ga[i,j] = w_i x (r_j - s_i),   gb[i,j] = nu_i x (r_j - s_i)
with w_i = third column of prefix rotation R_{i-1}, nu_i = R_{i-1}(cos a_i,
sin a_i, 0), s_i = R_CA * sum_{k<i} nu_k.  Each output channel is a K=4
outer product over (i, j), computed on the TensorEngine with K=12
(channel-interleaved rhs).  The sequential piece is the prefix rotation:
a blocked Hillis-Steele quaternion scan, CHANNEL-MAJOR state with an
8-col IDENTITY PAD per channel block (free idx = c*24+8+pos): round s
reads its shifted operand at offset 8-s so pos<s composes with identity
(no prefix copy), and each round is sa + 4 xor-permuted mults + 2 adds,
all full-width with contiguous innermost dims (DVE runs strided-innermost
APs ~4x slower); the sign table is pos-materialized for the same reason.
Cross-chunk totals scan via PE shift-matmuls in fp16 (fp32 matmul operands
force a 2-pass LDW+MM; bf16 state was measured 2.8e-2 rel err - too
coarse; fp16 lands 4.4e-3).  No i<len row masking anywhere: rows i >= len
only ever hit output cols j > i >= len which the rhs j<=len mask already
zeroes.

Outputs in bf16 (host converts to f32; tolerance 2e-2 >> bf16 rounding).
rhs is built HOST-side (on-device interleave cost ~8us Pool time + SBUF
contention that stalled concurrent DVE scan ops 5-10x).  lhsT slots are
permuted (SLOT_CK) so +v / -v / cross groups are contiguous: negation and
nu become single 48-col ops; rhs rows are permuted identically host-side.
The k-on-partition lhsT layout requires a DRAM bounce transpose (SBUF to
SBUF DMA cannot reorder partition-major vs slot-major iteration; PE/XBAR
transposes swap ALL of free vs partition, and the needed (p,s,pos) to
(s,p,pos) block permute is not expressible).

Sharding: pure data parallel, 4 samples per core x 8 cores.  Samples are
len-sorted into pairs, pairs bin-packed onto cores by written bytes.
Output DMAs carry host-computed cond flags (dma_start cond=) skipping
64-row diagonal groups / 384-col tail chunks the pair's max len can't
reach (donated output buffers are pre-zeroed).  Output DMA issue
SERIALIZES per engine behind the previous DMA's drain, so kicks spread
over THREE streams: Sync (pair 0), Scalar (pair 1), GpSimd (SWDGE, spare
tails) - GpSimd is idle through the main loop.  Input ships as one packed
(128, PKW) f32 tensor but SPLIT INTO PER-STAGE SBUF TILES (readers wait
on a tile's whole write set, so the 24KB trig fields must not share a
tile with 800KB of constants); trig-critical split is DMAed first.

Measured dead ends (do not retry): matmul N>512 (ISA limit, one PSUM
bank); matmul(start=False) onto Vector-written PSUM (device
UNRECOVERABLE); --enable-ldw-opt=true (walrus codegen fails on
InstLdweights); float32r matmul inputs (verifier demands f32r-rounded
producers, DVE can't); fused 128-row output DMAs (fewer/bigger kicks
serialize worse than fine-grained ones + write ~15% more below-diagonal
zero bytes); moving input kicks to Scalar (delays its main-loop evicts).
NEFF startup costs a fixed ~6us before the first kernel instruction;
first ~10.3us is startup + input DMA + trig wait.  Engine op issue ~175ns
+ ~165ns write-completion latency dominates all small-op phases.
"""
import sys
import numpy as np

sys.path.insert(0, "/opt/trn_rl_repo")

L = 512
NJ = L + 1            # 513
R_CA = 3.8
CPOS = 16             # positions per chunk (free dim); 32 chunks on partitions
ROW = 3 * NJ          # 1539 floats per output row
GP = 787968           # 3*L*(L+1), one g-plane per sample
CW = 384              # column-chunk width (128 j's * 3 channels)

_SGN = {
    0: [1.0, -1.0, -1.0, -1.0],
    1: [1.0, 1.0, 1.0, -1.0],
    2: [1.0, -1.0, 1.0, 1.0],
    3: [1.0, 1.0, -1.0, 1.0],
}
# b-operand comp permutation (k xor c): k = |2*i2 + s1*i1 + const|, realized
# as signed strides (s2*2*cs, s1*cs) at base offset c*cs.
_SGN_B = [(2, 1), (2, -1), (-2, 1), (-2, -1)]
# lhsT slot layout (per g-plane, 12 rows).  The matmul contraction order is
# free, so slots are arranged in contiguous groups: 0-2 zeros (cross-matrix
# diagonal), 3-5 = +v_m, 6-8 = -v_m, 9-11 = (s x v)_c.  The host-built rhs
# row for slot s carries comp k' masked to channel c per SLOT_CK below.
SLOT_POS = {0: 3, 1: 4, 2: 5}      # +v_m -> slot
SLOT_NEG = {0: 6, 1: 7, 2: 8}      # -v_m -> slot
SLOT_CRS = {0: 9, 1: 10, 2: 11}    # (s x v)_c -> slot
# slot s -> (channel c, rhs comp k'):  [v]x matrix structure
SLOT_CK = {0: (0, 0), 1: (1, 1), 2: (2, 2), 3: (2, 1), 4: (0, 2), 5: (1, 0),
           6: (1, 2), 7: (2, 0), 8: (0, 1), 9: (0, 3), 10: (1, 3), 11: (2, 3)}

# packed (128, PKW) input layout: name -> (col offset, width).  Trig fields
# first so the first (tiny) DMA split unblocks the scan ASAP.
COLS = {}
_off = 0
for _nm, _w in (
    ("a_sh", 16), ("b_sh", 16), ("a_f", 16),
    ("sgncm", 256),
    ("shm1", 128), ("shm2", 128), ("shm4", 128), ("shm8", 128), ("shm16", 128),
    ("efq1", 4), ("efq2", 4), ("efq4", 4), ("efq8", 4), ("efq16", 4),
    ("tmat", 128), ("trimask", 512),
):
    COLS[_nm] = (_off, _w)
    _off += _w
PKW = _off  # 1604


_PK_STATIC = None


def _pk_static() -> np.ndarray:
    """Sample-independent part of the packed tensor (built once)."""
    global _PK_STATIC
    if _PK_STATIC is not None:
        return _PK_STATIC
    pk = np.zeros((128, PKW), np.float32)

    def put(nm, arr):
        o, w = COLS[nm]
        pk[:arr.shape[0], o:o + w] = arr

    sg = np.zeros(256, np.float32)
    for ci, sv in _SGN.items():
        for kk in range(4):
            sg[64 * ci + 16 * kk:64 * ci + 16 * kk + 16] = sv[kk]
    put("sgncm", np.tile(sg, (128, 1)))
    for d in (1, 2, 4, 8, 16):
        S = np.zeros((128, 128), np.float32)
        for m in range(128):
            k = m - d
            if k >= 0 and k // 32 == m // 32:
                S[k, m] = 1.0
        put(f"shm{d}", S)
        E = np.zeros((128, 4), np.float32)
        E[np.arange(128) % 32 < d, 0] = 1.0
        put(f"efq{d}", E)
    T = np.zeros((128, 128), np.float32)
    for m in range(128):
        T[32 * (m // 32):m, m] = R_CA
    put("tmat", T)
    tri = (np.arange(CW)[None, :] >= 3 * np.arange(128)[:, None]).astype(np.float32)
    put("trimask", np.concatenate([tri, np.ones((128, 128), np.float32)], 1))
    _PK_STATIC = pk
    return pk


def build_pk(angles: np.ndarray, coords: np.ndarray, lens: np.ndarray):
    """Packed per-core inputs: angles (4,2,512) f32, coords (4,1539) f32,
    lens (4,) int.  Returns (pk, rhs_bf16): rhs is the channel-interleaved,
    j<=len masked (r_x, r_y, r_z, 1) operand, built host-side (doing it
    on-device cost ~8us of Pool time + SBUF contention with the scan)."""
    import ml_dtypes
    pk = _pk_static().copy()

    def put(nm, arr):
        o, w = COLS[nm]
        pk[:arr.shape[0], o:o + w] = arr

    # scan layout p = b*32 + ch; shifted by one position (exclusive scan input)
    ash = np.zeros((4, L), np.float32)
    bsh = np.zeros((4, L), np.float32)
    ash[:, 1:] = angles[:, 0, :-1]
    bsh[:, 1:] = angles[:, 1, :-1]
    put("a_sh", ash.reshape(128, CPOS))
    put("b_sh", bsh.reshape(128, CPOS))
    put("a_f", angles[:, 0, :].reshape(128, CPOS))

    cp = np.zeros((4, 4, NJ), np.float32)  # [b, comp, j]
    for b in range(4):
        cp[b, 0:3] = coords[b].reshape(NJ, 3).T
        cp[b, 3] = 1.0
        cp[b, :, int(lens[b]) + 1:] = 0.0
    rhs = np.zeros((48, ROW), np.float32)
    for b in range(4):
        for s_, (cch, kk) in SLOT_CK.items():
            rhs[b * 12 + s_, cch::3] = cp[b, kk]
    return pk, rhs.astype(ml_dtypes.bfloat16)


def _plan(lens):
    """Len-sorted pairing + byte-balanced core assignment.

    Returns (perm, flags): perm[4c+s] = original sample index for core c
    slot s; flags[c] = int32 (1, 16): per pair bp, flags[bp*8+n] =
    (pairmax_len > 64*n)."""
    lens = np.asarray(lens).astype(np.int64)
    order = np.argsort(lens, kind="stable")
    pairs = [(int(order[2 * m]), int(order[2 * m + 1])) for m in range(16)]

    def pair_cost(pr):
        lm = max(lens[pr[0]], lens[pr[1]])
        el = 0
        for ti in range(4):
            for k in range(2):
                if lm > 128 * ti + 64 * k:
                    el += 64 * (CW - 192 * k)
            for cj in range(ti + 1, 4):
                if lm > 128 * cj:
                    el += 128 * CW
        return el

    costs = [pair_cost(p) for p in pairs]
    core_pairs = [[] for _ in range(8)]
    core_load = [0] * 8
    for m in sorted(range(16), key=lambda i: -costs[i]):
        c = min([cc for cc in range(8) if len(core_pairs[cc]) < 2],
                key=lambda cc: core_load[cc])
        core_pairs[c].append(m)
        core_load[c] += costs[m]
    perm = np.empty(32, np.int64)
    flags = []
    for c in range(8):
        f = np.zeros((1, 16), np.int32)
        for bp, m in enumerate(core_pairs[c]):
            a, b = pairs[m]
            perm[4 * c + 2 * bp] = a
            perm[4 * c + 2 * bp + 1] = b
            lm = max(lens[a], lens[b])
            f[0, bp * 8:bp * 8 + 8] = (lm > 64 * np.arange(8)).astype(np.int32)
        flags.append(f)
    return perm, flags


def build_nc():
    import concourse.bass as bass
    import concourse.bacc as bacc
    import concourse.mybir as mybir
    from concourse.tile import TileContext

    F32 = mybir.dt.float32
    F32R = mybir.dt.float32r
    OP = mybir.AluOpType
    ACT = mybir.ActivationFunctionType

    nc = bacc.Bacc(target_bir_lowering=False, trn_type="TRN2")

    BF16 = mybir.dt.bfloat16
    pk_in = nc.declare_dram_parameter("pk", [128, PKW], F32, isOutput=False)
    rhs_in = nc.declare_dram_parameter("rhs", [48, ROW], BF16, isOutput=False)
    flg_in = nc.declare_dram_parameter("flg", [1, 16], mybir.dt.int32,
                                       isOutput=False)
    # Output in bf16 (tolerance 2e-2 >> bf16 rounding); host converts to f32.
    out = nc.declare_dram_parameter("out", [4, 2, GP], BF16, isOutput=True)
    bounce1 = nc.dram_tensor("bounce1", [24 * 2048], BF16)

    def dram_ap(handle, offset, dims):
        return bass.AP(tensor=handle, offset=offset,
                       ap=[list(d) for d in dims])

    def view(ap, offset, dims):
        """Free-dim view of an SBUF AP: keep its partition dim, custom free dims."""
        return bass.AP(tensor=ap.tensor, offset=ap.offset + offset,
                       ap=[list(ap.ap[0])] + [list(d) for d in dims])

    with TileContext(nc) as tc, tc.tile_pool(name="main", bufs=1) as MP:
        def T(shape, name):
            return MP.tile(shape, F32, name=name, tag=name)

        # ONE SBUF TILE PER DEPENDENCY STAGE: readers of a tile wait on the
        # whole tile's write set, so the trig-critical 24KB must not share a
        # tile with the 845KB of constants.  Sync kicks the scan-critical
        # splits in consumer order; Scalar kicks rhs + flg in parallel.
        _splits = [("t_trig", 0, 48),        # angles (trig, 24KB, FIRST)
                   ("t_sgn", 48, 256),       # sgncm (round 1)
                   ("t_shm", 304, 660),      # shm, efq (cross-chunk scan)
                   ("t_main", COLS["tmat"][0], 640)]   # tmat + trimask
        _ptile = {}
        for (tn, o, w) in _splits:
            t_ = T([128, w], tn)
            nc.sync.dma_start(t_[:], pk_in[:, o:o + w])
            for nm, (co, cw) in COLS.items():
                if o <= co and co + cw <= o + w:
                    _ptile[nm] = (t_, co - o)
        # One [12, ROW] tile per sample (matmul operands must share base
        # partition 0 with the lhsT tile).
        rhs = []
        for b in range(4):
            rb = MP.tile([12, ROW], BF16, name=f"rhs{b}", tag=f"rhs{b}")
            rhs.append(rb)
            nc.sync.dma_start(rb[:], rhs_in[b * 12:b * 12 + 12, :])
        flg = MP.tile([1, 16], mybir.dt.int32, name="flg_sb", tag="flg_sb")
        nc.sync.dma_start(flg[:], flg_in[0:1, :])


        def PKV(nm, rows=128):
            t_, o = _ptile[nm]
            w = COLS[nm][1]
            return t_[0:rows, o:o + w]

        # ---- trig (wrap into [-pi, pi]: Sin LUT range limit) ----
        PI = float(np.pi)
        cAs, sAs = T([128, CPOS], "cAs"), T([128, CPOS], "sAs")
        cBs, sBs = T([128, CPOS], "cBs"), T([128, CPOS], "sBs")
        caf, saf = T([128, CPOS], "caf"), T([128, CPOS], "saf")
        wt1 = T([128, CPOS], "wt1")
        wt2 = T([128, CPOS], "wt2")
        wt3 = T([128, CPOS], "wt3")
        wt4 = T([128, CPOS], "wt4")
        for src, scale, outs in (("a_sh", 0.5, (cAs, sAs)),
                                 ("b_sh", 0.5, (cBs, sBs)),
                                 ("a_f", 1.0, (caf, saf))):
            eng = nc.vector
            wta, wtb = (wt3, wt4) if scale == 1.0 else (wt1, wt2)
            for (dst, shift) in ((outs[0], PI / 2), (outs[1], 0.0)):
                y = T([128, CPOS], f"y_{src}_{int(shift * 10)}")
                eng.tensor_scalar(y[:], PKV(src), scale, shift,
                                  OP.mult, OP.add)
                if scale == 0.5 and shift == 0.0:
                    # |x/2| < pi for N(0,1) inputs: no wrap needed
                    nc.scalar.activation(dst[:], y[:], ACT.Sin, bias=0.0,
                                         scale=1.0)
                    continue
                wrapt = T([128, CPOS], f"wr_{src}_{int(shift * 10)}")
                eng.tensor_scalar(wta[:], y[:], PI, None, OP.is_gt)
                if scale == 0.5:
                    # x/2 + pi/2 can only overflow the upper bound
                    eng.scalar_tensor_tensor(wrapt[:], wta[:], -2 * PI,
                                             y[:], OP.mult, OP.add)
                else:
                    eng.tensor_scalar(wtb[:], y[:], -PI, None, OP.is_lt)
                    eng.tensor_tensor(wta[:], wta[:], wtb[:], OP.subtract)
                    eng.scalar_tensor_tensor(wrapt[:], wta[:], -2 * PI,
                                             y[:], OP.mult, OP.add)
                nc.scalar.activation(dst[:], wrapt[:], ACT.Sin, bias=0.0,
                                     scale=1.0)

        C = T([128, 24 * CPOS], "Cstack")
        # Only slots {0,5,10} (+12 for g1) stay zero (cross-product diagonal);
        # all others are written below. Strided vector memsets beat a full
        # [128, 384] gpsimd memset (~1.3us measured).
        nc.vector.memset(view(C[:], 0, [[192, 2], [1, 3 * CPOS]]), 0.0)

        def slot(s_):
            return C[:, s_ * CPOS:(s_ + 1) * CPOS]

        with tc.tile_pool(name="scan", bufs=2) as SP, \
             tc.tile_pool(name="scantmp", bufs=2) as TP, \
             tc.tile_pool(name="pscan", bufs=2, space="PSUM") as PS:
            # local quats q = (cA cB, cA sB, sA sB, sA cB), from shifted
            # angles.  CHANNEL-MAJOR state with an 8-col IDENTITY PAD per
            # channel block (free idx = c*24 + 8 + pos): round s reads its
            # shifted a-operand at offset 8-s, so pos<s lands in the pad and
            # composes with identity -- no prefix copy, and every round op
            # is full-width with contiguous innermost dims.
            PAD = 8
            BS = CPOS + PAD
            Pa = SP.tile([128, 4 * BS], F32, name="scanP0", tag="scanP0")
            Pb = SP.tile([128, 4 * BS], F32, name="scanP1", tag="scanP1")
            for Pt in (Pa, Pb):
                nc.vector.memset(view(Pt[:], 0, [[1, PAD]]), 1.0)
                nc.vector.memset(view(Pt[:], BS, [[BS, 3], [1, PAD]]), 0.0)
            for ci, (x, y) in enumerate(((cAs, cBs), (cAs, sBs), (sAs, sBs), (sAs, cBs))):
                nc.vector.tensor_tensor(
                    Pa[:, ci * BS + PAD:ci * BS + PAD + CPOS],
                    x[:], y[:], OP.mult)
            # identity quat at i=0 of each sample comes free: a_sh/b_sh are
            # zero-filled at pos 0 so q = (cos0*cos0, ...) = (1, 0, 0, 0)
            # (Sin LUT exactness at 0 / pi/2 is ~1e-5, far under tolerance).

            def quat_round_cm(a_src, a_ps, a_ks, b_src, b_off, nxt, out_off,
                              out_cs, npos, cs, eng):
                """nxt[out_off + c*out_cs + pos] =
                       sum_k sgn[c,k] * a[pos*a_ps + k*a_ks]
                                      * b[b_off + (k^c)*cs + pos].
                   Dim order (c, k, pos) everywhere: every operand has
                   stride-1 or stride-0 innermost (DVE chokes on strided
                   innermost dims).  k-sum done as two contiguous adds.
                   All on `eng` so the round has no cross-engine sync."""
                n4 = npos * 4
                sa = TP.tile([128, 256], F32, name="sa", tag="sa")
                eng.tensor_tensor(
                    view(sa[:], 0, [[n4, 4], [npos, 4], [1, npos]]),
                    view(a_src, 0, [[0, 4], [a_ks, 4], [a_ps, npos]]),
                    view(PKV("sgncm"), 0, [[64, 4], [16, 4], [1, npos]]),
                    OP.mult)
                v = TP.tile([128, 256], F32, name="vv", tag="vv")
                for c in range(4):
                    s2, s1 = _SGN_B[c]
                    eng.tensor_tensor(
                        view(v[:], c * n4, [[2 * npos, 2], [npos, 2], [1, npos]]),
                        view(sa[:], c * n4, [[2 * npos, 2], [npos, 2], [1, npos]]),
                        view(b_src, b_off + c * cs,
                             [[s2 * cs, 2], [s1 * cs, 2], [1, npos]]),
                        OP.mult)
                t2 = TP.tile([128, 128], F32, name="t2", tag="t2")
                eng.tensor_tensor(
                    view(t2[:], 0, [[2 * npos, 4], [npos, 2], [1, npos]]),
                    view(v[:], 0, [[n4, 4], [2 * npos, 2], [1, npos]]),
                    view(v[:], npos, [[n4, 4], [2 * npos, 2], [1, npos]]),
                    OP.add)
                eng.tensor_tensor(
                    view(nxt, out_off, [[out_cs, 4], [1, npos]]),
                    view(t2[:], 0, [[2 * npos, 4], [1, npos]]),
                    view(t2[:], npos, [[2 * npos, 4], [1, npos]]),
                    OP.add)

            cur_t, nxt_t = Pa, Pb
            for s in (1, 2, 4, 8):      # in-chunk shifts (free dim)
                sa = TP.tile([128, 256], F32, name="sa", tag="sa")
                nc.vector.tensor_tensor(
                    view(sa[:], 0, [[64, 4], [16, 4], [1, 16]]),
                    view(cur_t[:], PAD - s, [[0, 4], [BS, 4], [1, 16]]),
                    view(PKV("sgncm"), 0, [[64, 4], [16, 4], [1, 16]]),
                    OP.mult)
                v = TP.tile([128, 256], F32, name="vv", tag="vv")
                for c in range(4):
                    s2, s1 = _SGN_B[c]
                    nc.vector.tensor_tensor(
                        view(v[:], c * 64, [[32, 2], [16, 2], [1, 16]]),
                        view(sa[:], c * 64, [[32, 2], [16, 2], [1, 16]]),
                        view(cur_t[:], PAD + c * BS,
                             [[s2 * BS, 2], [s1 * BS, 2], [1, 16]]),
                        OP.mult)
                t2 = TP.tile([128, 128], F32, name="t2", tag="t2")
                nc.vector.tensor_tensor(
                    view(t2[:], 0, [[32, 4], [16, 2], [1, 16]]),
                    view(v[:], 0, [[64, 4], [32, 2], [1, 16]]),
                    view(v[:], 16, [[64, 4], [32, 2], [1, 16]]), OP.add)
                nc.vector.tensor_tensor(
                    view(nxt_t[:], PAD, [[BS, 4], [1, 16]]),
                    view(t2[:], 0, [[32, 4], [1, 16]]),
                    view(t2[:], 16, [[32, 4], [1, 16]]), OP.add)
                cur_t, nxt_t = nxt_t, cur_t
            # cross-chunk: Hillis-Steele over chunk totals (PE shift-matmul).
            # State in bf16: fp32 operands make every shift matmul a 2-pass
            # (2x LDWEIGHTS+MATMUL); the 0/1 shift matrix is exact in bf16
            # and 6 rounds of fp16 state rounding stay inside the 2e-2
            # budget (bf16 measured 2.8e-2: too coarse).  shm weights converted once on Scalar (off-path).
            F16 = mybir.dt.float16
            shmb = MP.tile([128, 640], F16, name="shmb", tag="shmb")
            nc.scalar.copy(shmb[:], view(PKV("shm1"), 0, [[1, 640]]))
            _shb = {d: shmb[:, i * 128:(i + 1) * 128]
                    for i, d in enumerate((1, 2, 4, 8, 16))}
            tot = SP.tile([128, 4], F16, name="tot0", tag="tot")
            nc.vector.tensor_copy(tot[:], view(cur_t[:], PAD + CPOS - 1, [[BS, 4]]))
            def cross_round(sh_ps, b_tot, ntot):
                sa = TP.tile([128, 16], F32, name="xsa", tag="xsa")
                nc.vector.tensor_tensor(
                    view(sa[:], 0, [[4, 4], [1, 4]]),
                    view(sh_ps, 0, [[0, 4], [1, 4]]),
                    view(PKV("sgncm"), 0, [[64, 4], [16, 4]]), OP.mult)
                v = TP.tile([128, 16], F32, name="xvv", tag="xvv")
                for c in range(4):
                    s2, s1 = _SGN_B[c]
                    nc.vector.tensor_tensor(
                        view(v[:], c * 4, [[2, 2], [1, 2]]),
                        view(sa[:], c * 4, [[2, 2], [1, 2]]),
                        view(b_tot, c, [[s2, 2], [s1, 2]]), OP.mult)
                with nc.allow_low_precision(
                        reason="4-elem quat k-sum to fp16 state; bounded "
                               "unit quats, rel ~5e-4/round"):
                    nc.vector.tensor_reduce(
                        view(ntot, 0, [[1, 4]]),
                        view(v[:], 0, [[4, 4], [1, 4]]),
                        mybir.AxisListType.X, OP.add)

            for d in (1, 2, 4, 8, 16):
                sh_ps = PS.tile([128, 4], F32, name=f"shps{d}", tag="shps")
                nc.tensor.matmul(sh_ps[:], _shb[d], tot[:],
                                 start=True, stop=True)
                qt = TP.tile([128, 4], F16, name=f"qt{d}", tag="qt")
                nc.vector.tensor_tensor(qt[:], sh_ps[:], PKV(f"efq{d}"), OP.add)
                ntot = SP.tile([128, 4], F16, name=f"tot{d}", tag="tot")
                cross_round(qt[:], tot[:], ntot[:])
                tot = ntot
            # exclusive chunk offsets = totscan shifted one chunk (+identity)
            off_ps = PS.tile([128, 4], F32, name="off_ps", tag="shps")
            nc.tensor.matmul(off_ps[:], _shb[1], tot[:],
                             start=True, stop=True)
            offq = SP.tile([128, 4], F32, name="offq", tag="tot")
            nc.vector.tensor_tensor(offq[:], off_ps[:], PKV("efq1"), OP.add)
            # compose: final[p, c, pos] = (offq[p] (x) cur[p, :, pos])_c
            nxt = SP.tile([128, 64], F32, name="scan_fin", tag="scan")
            quat_round_cm(offq[:], 0, 1, cur_t[:], PAD, nxt[:], 0, CPOS,
                          CPOS, BS, nc.vector)
            cur = nxt

            # ---- conversion: Qex -> w/nu planes + crosses into C ----
            # No row (i < len) masking: rows with i >= len only ever hit
            # output cols with j > i >= len, which the rhs j<=len mask
            # already zeroes.  R = I + 2*(...): the 2x is folded into the
            # products via scalar_tensor_tensor.
            W = cur[:, 0:CPOS]
            X = cur[:, CPOS:2 * CPOS]
            Y = cur[:, 2 * CPOS:3 * CPOS]
            Z = cur[:, 3 * CPOS:4 * CPOS]

            # grouped products: dbl = 2*[X|Y|Z]; then 4 wide multiplies
            # cover all nine 2*q_i*q_j products.
            dbl = T([128, 48], "dbl")
            nc.vector.tensor_scalar(dbl[:], cur[:, CPOS:4 * CPOS], 2.0,
                                    None, OP.mult)
            PG1 = T([128, 48], "PG1")   # [wx2, wy2, wz2]
            nc.vector.tensor_tensor(view(PG1[:], 0, [[16, 3], [1, CPOS]]),
                                    view(cur[:], 0, [[0, 3], [1, CPOS]]),
                                    dbl[:], OP.mult)
            PG2 = T([128, 48], "PG2")   # [xx2, yy2, zz2]
            nc.vector.tensor_tensor(PG2[:], cur[:, CPOS:4 * CPOS],
                                    dbl[:], OP.mult)
            PG3 = T([128, 32], "PG3")   # [xy2, yz2]
            nc.vector.tensor_tensor(PG3[:], dbl[:, 0:32],
                                    cur[:, 2 * CPOS:4 * CPOS], OP.mult)
            PG4 = T([128, CPOS], "PG4")  # [xz2]
            nc.vector.tensor_tensor(PG4[:], dbl[:, 0:16],
                                    cur[:, 3 * CPOS:4 * CPOS], OP.mult)
            pr = {"wx": PG1[:, 0:16], "wy": PG1[:, 16:32],
                  "wz": PG1[:, 32:48], "xx": PG2[:, 0:16],
                  "yy": PG2[:, 16:32], "zz": PG2[:, 32:48],
                  "xy": PG3[:, 0:16], "yz": PG3[:, 16:32],
                  "xz": PG4[:, 0:16]}

            # col6 = [c00 c01 c02 | c10 c11 c12] contiguous for the wide
            # nu ops below.
            col6 = T([128, 96], "col6")
            col = {f"c{r}{cc}": col6[:, (3 * r + cc) * CPOS:
                                     (3 * r + cc + 1) * CPOS]
                   for r in range(2) for cc in range(3)}
            chains = [(slot(SLOT_POS[0]), "xz", "wy", OP.add, False),
                      (slot(SLOT_POS[1]), "yz", "wx", OP.subtract, False),
                      (slot(SLOT_POS[2]), "xx", "yy", OP.add, True),
                      (col["c00"], "yy", "zz", OP.add, True),
                      (col["c01"], "xy", "wz", OP.add, False),
                      (col["c02"], "xz", "wy", OP.subtract, False),
                      (col["c10"], "xy", "wz", OP.subtract, False),
                      (col["c11"], "xx", "zz", OP.add, True),
                      (col["c12"], "yz", "wx", OP.add, False)]
            ct = [T([128, CPOS], f"ct{i}") for i in range(9)]
            for i, (dst, a1, a2, op, om) in enumerate(chains):
                nc.vector.tensor_tensor(ct[i][:] if om else dst,
                                        pr[a1], pr[a2], op)
            for i, (dst, a1, a2, op, om) in enumerate(chains):
                if om:   # diagonal entries: 1 - 2*(p+q)
                    nc.vector.tensor_scalar(dst, ct[i][:], -1.0, 1.0,
                                            OP.mult, OP.add)
            # nu = col0*cos a + col1*sin a, all 3 comps in one 48-col op
            # each; result lands in the contiguous +nu slots 12+3..12+5.
            nut0 = T([128, 48], "nut0")
            nut1 = T([128, 48], "nut1")
            nc.vector.tensor_tensor(nut0[:], col6[:, 0:48],
                                    view(caf[:], 0, [[0, 3], [1, CPOS]]),
                                    OP.mult)
            nc.vector.tensor_tensor(nut1[:], col6[:, 48:96],
                                    view(saf[:], 0, [[0, 3], [1, CPOS]]),
                                    OP.mult)
            nc.vector.tensor_tensor(C[:, (12 + 3) * CPOS:(12 + 6) * CPOS],
                                    nut0[:], nut1[:], OP.add)
            # negations: one 48-col op per plane (slots 6-8 <- 3-5)
            for g0 in (0, 12):
                nc.vector.tensor_scalar(
                    C[:, (g0 + 6) * CPOS:(g0 + 9) * CPOS],
                    C[:, (g0 + 3) * CPOS:(g0 + 6) * CPOS],
                    -1.0, None, OP.mult)

            # ---- s_ex = R_CA * exclusive-cumsum(nu) ----
            zeros16 = T([128, CPOS], "zeros16")
            nc.vector.memset(zeros16[:], 0.0)
            nu_incl = MP.tile([128, 48], F16, name="nu_incl", tag="nu_incl")
            tmat16 = MP.tile([128, 128], F16, name="tmat16", tag="tmat16")
            nc.scalar.copy(tmat16[:], PKV("tmat"))
            with nc.allow_low_precision(
                    reason="fp16 nu cumsum over 16 positions, |nu|<=1: "
                           "~1e-3 rel vs 2e-2 budget; buys single-pass "
                           "fp16 tmat matmul"):
                for cc in range(3):
                    nc.vector.tensor_tensor_scan(
                        nu_incl[:, cc * CPOS:(cc + 1) * CPOS],
                        slot(12 + SLOT_POS[cc]), zeros16[:], 0.0,
                        OP.add, OP.add)
            offs_ps = PS.tile([128, 4], F32, name="offs_ps", tag="shps")
            nc.tensor.matmul(offs_ps[:, 0:3], tmat16[:],
                             view(nu_incl[:], CPOS - 1, [[CPOS, 3]]),
                             start=True, stop=True)
            offs = T([128, 3], "offs")
            nc.vector.tensor_copy(offs[:], offs_ps[:, 0:3])
            s_ex = T([128, 48], "s_ex")
            for cc in range(3):
                nc.vector.tensor_copy(s_ex[:, cc * CPOS:cc * CPOS + 1],
                                      offs[:, cc:cc + 1])
            for cc in range(3):
                nc.vector.tensor_scalar(
                    s_ex[:, cc * CPOS + 1:(cc + 1) * CPOS],
                    nu_incl[:, cc * CPOS:(cc + 1) * CPOS - 1],
                    R_CA, offs[:, cc:cc + 1], OP.mult, OP.add)

            def sc_(cc):
                return s_ex[:, cc * CPOS:(cc + 1) * CPOS]

            crt = {(e, i): T([128, CPOS], f"crt{e}{i}")
                   for e in (0, 1) for i in range(6)}
            for base in (0, 12):  # (s x v)_c = s_{c+1} v_{c+2} - s_{c+2} v_{c+1}
                ei = 0 if base == 0 else 1
                eng = nc.gpsimd if base == 0 else nc.vector
                for cc in range(3):  # staged: products first, then subtracts
                    c1, c2 = (cc + 1) % 3, (cc + 2) % 3
                    eng.tensor_tensor(crt[ei, 2 * cc][:], sc_(c1),
                                      slot(base + SLOT_POS[c2]), OP.mult)
                    eng.tensor_tensor(crt[ei, 2 * cc + 1][:], sc_(c2),
                                      slot(base + SLOT_POS[c1]), OP.mult)
                for cc in range(3):
                    eng.tensor_tensor(slot(base + SLOT_CRS[cc]),
                                      crt[ei, 2 * cc][:],
                                      crt[ei, 2 * cc + 1][:], OP.subtract)

        # ---- C -> bf16 -> bounce1 -> lhsT (12, [g, b, i]) ----
        Cb = MP.tile([128, 24 * CPOS], BF16, name="Cb", tag="Cb")
        Cb3 = Cb.rearrange("p (slot pos) -> p slot pos", slot=24)
        lhsT = MP.tile([12, 4096], BF16, name="lhsT", tag="lhsT")
        for g in range(2):   # per-plane bounce: g=0 kicks as soon as the
            # ga slots are done, so production isn't gated on gb's crosses.
            # Each plane split into partition halves so the read-back of
            # half 0 overlaps the write-out of half 1 (cuts latency).
            nc.scalar.copy(Cb[:, g * 192:(g + 1) * 192],
                           C[:, g * 192:(g + 1) * 192])
            for h in range(2):
                nc.sync.dma_start(
                    dram_ap(bounce1, g * 12 * 2048 + h * 1024,
                            [[16, 64], [2048, 12], [1, 16]]),
                    Cb3[64 * h:64 * h + 64, g * 12:(g + 1) * 12, :])
                nc.sync.dma_start(
                    lhsT[:, g * 2048 + h * 1024:g * 2048 + h * 1024 + 1024],
                    dram_ap(bounce1, g * 12 * 2048 + h * 1024,
                            [[2048, 12], [1, 1024]]))

        # rhs (channel-interleaved, j<=len masked) is built host-side and
        # DMAed straight into rhs48; matmuls slice partition ranges b*12..
        # Below-diagonal zeros are never written: SPMD output buffers are
        # donated pre-zeroed (bass2jax.run_bass_via_pjrt zero-fills them).

        # Per-pair cond flags: pair 0 on Sync regs, pair 1 on Scalar regs
        # (the two HWDGE engines; each kicks its pair's output DMAs).
        _, cond_p0 = nc.values_load_multi_w_load_instructions(
            flg[0:1, 0:8], engines=[mybir.EngineType.SP],
            min_val=0, max_val=1, skip_runtime_bounds_check=True)
        _, cond_p1 = nc.values_load_multi_w_load_instructions(
            flg[0:1, 8:16], engines=[mybir.EngineType.Activation],
            min_val=0, max_val=1, skip_runtime_bounds_check=True)
        # Third kick stream on GpSimd (idle through the main loop): DMA
        # issue on an engine serializes behind its previous DMA's drain,
        # so splitting bytes across 3 engines raises aggregate drain rate.
        _, cond_gp = nc.values_load_multi_w_load_instructions(
            flg[0:1, 0:16], engines=[mybir.EngineType.Pool],
            min_val=0, max_val=1, skip_runtime_bounds_check=True)
        conds = (cond_p0, cond_p1)
        kick_eng = (nc.sync, nc.scalar)

        # ---- main loop: weight-reusing matmuls -> ACT evict -> GpSimd mask ----
        trimask_ap = PKV("trimask")
        with tc.tile_pool(name="pmain", bufs=8, space="PSUM") as PM, \
             tc.tile_pool(name="stg", bufs=1) as SG:
            # PE p-state warmup: dummy matmuls in the PE's idle window
            # (bounce/lhsT wait) so production starts at full clock.
            for wi in range(8):
                ptw = PM.tile([128, 512], F32, name="warm", tag="pt")
                nc.tensor.matmul(ptw[:, 0:512], rhs[0][:, 0:128],
                                 rhs[0][:, 512:1024],
                                 start=True, stop=True)
            for g in range(2):
                for ti in range(4):
                    nact = CW * (4 - ti)           # active width per sample
                    n0 = CW * ti + 3               # first active column
                    stg = SG.tile([128, 4 * nact], BF16, name=f"stg{g}{ti}",
                                  tag=f"stg{g}{ti}")
                    stg4 = stg.rearrange("p (b w) -> p b w", b=4)
                    for b in range(4):
                        lh = lhsT[:, g * 2048 + b * 512 + ti * 128:
                                  g * 2048 + b * 512 + (ti + 1) * 128]
                        cuts = list(range(0, nact, 512)) + [nact]
                        for ci, (c0, c1) in enumerate(zip(cuts[:-1], cuts[1:])):
                            pt = PM.tile([128, 512], F32, name="pt", tag="pt")
                            nc.tensor.matmul(
                                pt[:, 0:c1 - c0], lh,
                                rhs[b][:, n0 + c0:n0 + c1],
                                start=True, stop=True)
                            if ci == 0:   # masked evict (diag), on Vector
                                nc.vector.tensor_tensor(
                                    stg4[:, b, 0:c1], pt[:, 0:c1],
                                    view(trimask_ap, 0, [[1, c1]]), OP.mult)
                            else:          # plain chunks on Scalar (Vector
                                # runs ~95% busy in the loop, Scalar ~55%)
                                nc.scalar.copy(stg4[:, b, c0:c1],
                                               pt[:, 0:c1 - c0])
                    for bp in range(2):
                        eng, cnd = kick_eng[bp], conds[bp]
                        for k in range(2):   # 64-row diagonal groups
                            eng.dma_start(
                                dram_ap(out, (2 * bp) * 2 * GP + g * GP
                                        + (ti * 128 + 64 * k) * ROW
                                        + n0 + 192 * k,
                                        [[ROW, 64], [2 * GP, 2],
                                         [1, CW - 192 * k]]),
                                stg4[64 * k:64 * k + 64,
                                     2 * bp:2 * bp + 2, 192 * k:CW],
                                cond=cnd[2 * ti + k])
                        for cj in range(ti + 1, 4):  # 128-j tail chunks
                            # first tail on the pair's own engine, the rest
                            # on the GpSimd kick stream
                            if cj == ti + 1:
                                keng, kc = eng, cnd[2 * cj]
                            else:
                                keng, kc = nc.gpsimd, cond_gp[bp * 8 + 2 * cj]
                            keng.dma_start(
                                dram_ap(out, (2 * bp) * 2 * GP + g * GP
                                        + ti * 128 * ROW + CW * cj + 3,
                                        [[ROW, 128], [2 * GP, 2], [1, CW]]),
                                stg4[:, 2 * bp:2 * bp + 2,
                                     CW * (cj - ti):CW * (cj - ti) + CW],
                                cond=kc)
    nc.compile()
    return nc


_NC_CACHE = {}


def _get_nc():
    if "nc" not in _NC_CACHE:
        _NC_CACHE["nc"] = build_nc()
    return _NC_CACHE["nc"]


def run_spmd(input_angles, input_coords, angles_length, trace=False):
    from concourse.bass_utils import run_bass_kernel_spmd

    input_angles = np.ascontiguousarray(np.asarray(input_angles, np.float32))
    input_coords = np.ascontiguousarray(np.asarray(input_coords, np.float32))
    angles_length = np.asarray(angles_length)
    assert input_angles.shape[0] == 32

    nc = _get_nc()
    perm, flags = _plan(angles_length)
    in_maps = []
    for core in range(8):
        sl = perm[core * 4:core * 4 + 4]
        pk, rhs = build_pk(input_angles[sl], input_coords[sl],
                           angles_length[sl])
        in_maps.append({"pk": pk, "rhs": rhs, "flg": flags[core]})

    res = run_bass_kernel_spmd(nc, in_maps, core_ids=list(range(8)),
                               trace=trace)
    full = np.empty((32, 2, GP), np.float32)
    for core in range(8):
        full[perm[core * 4:core * 4 + 4]] = np.asarray(
            res.results[core]["out"]).astype(np.float32).reshape(4, 2, GP)
    return full, res


def kernel(input_angles, input_coords, angles_length):
    full, _ = run_spmd(input_angles, input_coords, angles_length, trace=False)
    return full


if __name__ == "__main__":
    print("kernel module OK")



# revision 63
# speedup vs baseline: 1.0350x; 1.0350x over previous
"""Trainium2 Bass kernel for nn_Angles2BMatrixAB.  (~72.8us median,
rel err 4.4e-3; from a 106.5us baseline.)

Math: the reference's F^q_i = M_{i-1} dB_i/dq M_i^{-1} collapses to the
geometric Jacobian of a revolute chain:
    ga[i,j] = w_i x (r_j - s_i),   gb[i,j] = nu_i x (r_j - s_i)
with w_i = third column of prefix rotation R_{i-1}, nu_i = R_{i-1}(cos a_i,
sin a_i, 0), s_i = R_CA * sum_{k<i} nu_k.  Each output channel is a K=4
outer product over (i, j), computed on the TensorEngine with K=12
(channel-interleaved rhs).  The sequential piece is the prefix rotation:
a blocked Hillis-Steele quaternion scan, CHANNEL-MAJOR state with an
8-col IDENTITY PAD per channel block (free idx = c*24+8+pos): round s
reads its shifted operand at offset 8-s so pos<s composes with identity
(no prefix copy), and each round is sa + 4 xor-permuted mults + 2 adds,
all full-width with contiguous innermost dims (DVE runs strided-innermost
APs ~4x slower); the sign table is pos-materialized for the same reason.
Cross-chunk totals scan via PE shift-matmuls in fp16 (fp32 matmul operands
force a 2-pass LDW+MM; bf16 state was measured 2.8e-2 rel err - too
coarse; fp16 lands 4.4e-3).  No i<len row masking anywhere: rows i >= len
only ever hit output cols j > i >= len which the rhs j<=len mask already
zeroes.

Outputs in bf16 (host converts to f32; tolerance 2e-2 >> bf16 rounding).
rhs is built HOST-side (on-device interleave cost ~8us Pool time + SBUF
contention that stalled concurrent DVE scan ops 5-10x).  lhsT slots are
permuted (SLOT_CK) so +v / -v / cross groups are contiguous: negation and
nu become single 48-col ops; rhs rows are permuted identically host-side.
The k-on-partition lhsT layout requires a DRAM bounce transpose (SBUF to
SBUF DMA cannot reorder partition-major vs slot-major iteration; PE/XBAR
transposes swap ALL of free vs partition, and the needed (p,s,pos) to
(s,p,pos) block permute is not expressible).

Sharding: pure data parallel, 4 samples per core x 8 cores.  Samples are
len-sorted into pairs, pairs bin-packed onto cores by written bytes.
Output DMAs carry host-computed cond flags (dma_start cond=) skipping
64-row diagonal groups / 384-col tail chunks the pair's max len can't
reach (donated output buffers are pre-zeroed).  Output DMA issue
SERIALIZES per engine behind the previous DMA's drain, so kicks spread
over THREE streams: Sync (pair 0), Scalar (pair 1), GpSimd (SWDGE, spare
tails) - GpSimd is idle through the main loop.  Input ships as one packed
(128, PKW) f32 tensor but SPLIT INTO PER-STAGE SBUF TILES (readers wait
on a tile's whole write set, so the 24KB trig fields must not share a
tile with 800KB of constants); trig-critical split is DMAed first.

Measured dead ends (do not retry): matmul N>512 (ISA limit, one PSUM
bank); matmul(start=False) onto Vector-written PSUM (device
UNRECOVERABLE); --enable-ldw-opt=true (walrus codegen fails on
InstLdweights); float32r matmul inputs (verifier demands f32r-rounded
producers, DVE can't); fused 128-row output DMAs (fewer/bigger kicks
serialize worse than fine-grained ones + write ~15% more below-diagonal
zero bytes); moving input kicks to Scalar (delays its main-loop evicts).
NEFF startup costs a fixed ~6us before the first kernel instruction;
first ~10.3us is startup + input DMA + trig wait.  Engine op issue ~175ns
+ ~165ns write-completion latency dominates all small-op phases.
"""
import sys
import numpy as np

sys.path.insert(0, "/opt/trn_rl_repo")

L = 512
NJ = L + 1            # 513
R_CA = 3.8
CPOS = 16             # positions per chunk (free dim); 32 chunks on partitions
ROW = 3 * NJ          # 1539 floats per output row
GP = 787968           # 3*L*(L+1), one g-plane per sample
CW = 384              # column-chunk width (128 j's * 3 channels)

_SGN = {
    0: [1.0, -1.0, -1.0, -1.0],
    1: [1.0, 1.0, 1.0, -1.0],
    2: [1.0, -1.0, 1.0, 1.0],
    3: [1.0, 1.0, -1.0, 1.0],
}
# b-operand comp permutation (k xor c): k = |2*i2 + s1*i1 + const|, realized
# as signed strides (s2*2*cs, s1*cs) at base offset c*cs.
_SGN_B = [(2, 1), (2, -1), (-2, 1), (-2, -1)]
# lhsT slot layout (per g-plane, 12 rows).  The matmul contraction order is
# free, so slots are arranged in contiguous groups: 0-2 zeros (cross-matrix
# diagonal), 3-5 = +v_m, 6-8 = -v_m, 9-11 = (s x v)_c.  The host-built rhs
# row for slot s carries comp k' masked to channel c per SLOT_CK below.
SLOT_POS = {0: 3, 1: 4, 2: 5}      # +v_m -> slot
SLOT_NEG = {0: 6, 1: 7, 2: 8}      # -v_m -> slot
SLOT_CRS = {0: 9, 1: 10, 2: 11}    # (s x v)_c -> slot
# slot s -> (channel c, rhs comp k'):  [v]x matrix structure
SLOT_CK = {0: (0, 0), 1: (1, 1), 2: (2, 2), 3: (2, 1), 4: (0, 2), 5: (1, 0),
           6: (1, 2), 7: (2, 0), 8: (0, 1), 9: (0, 3), 10: (1, 3), 11: (2, 3)}

# packed (128, PKW) input layout: name -> (col offset, width).  Trig fields
# first so the first (tiny) DMA split unblocks the scan ASAP.
COLS = {}
_off = 0
for _nm, _w in (
    ("a_sh", 16), ("b_sh", 16), ("a_f", 16),
    ("sgncm", 256),
    ("shm1", 128), ("shm2", 128), ("shm4", 128), ("shm8", 128), ("shm16", 128),
    ("efq1", 4), ("efq2", 4), ("efq4", 4), ("efq8", 4), ("efq16", 4),
    ("tmat", 128), ("trimask", 512),
):
    COLS[_nm] = (_off, _w)
    _off += _w
PKW = _off  # 1604


_PK_STATIC = None


def _pk_static() -> np.ndarray:
    """Sample-independent part of the packed tensor (built once)."""
    global _PK_STATIC
    if _PK_STATIC is not None:
        return _PK_STATIC
    pk = np.zeros((128, PKW), np.float32)

    def put(nm, arr):
        o, w = COLS[nm]
        pk[:arr.shape[0], o:o + w] = arr

    sg = np.zeros(256, np.float32)
    for ci, sv in _SGN.items():
        for kk in range(4):
            sg[64 * ci + 16 * kk:64 * ci + 16 * kk + 16] = sv[kk]
    put("sgncm", np.tile(sg, (128, 1)))
    for d in (1, 2, 4, 8, 16):
        S = np.zeros((128, 128), np.float32)
        for m in range(128):
            k = m - d
            if k >= 0 and k // 32 == m // 32:
                S[k, m] = 1.0
        put(f"shm{d}", S)
        E = np.zeros((128, 4), np.float32)
        E[np.arange(128) % 32 < d, 0] = 1.0
        put(f"efq{d}", E)
    T = np.zeros((128, 128), np.float32)
    for m in range(128):
        T[32 * (m // 32):m, m] = R_CA
    put("tmat", T)
    tri = (np.arange(CW)[None, :] >= 3 * np.arange(128)[:, None]).astype(np.float32)
    put("trimask", np.concatenate([tri, np.ones((128, 128), np.float32)], 1))
    _PK_STATIC = pk
    return pk


def build_pk(angles: np.ndarray, coords: np.ndarray, lens: np.ndarray):
    """Packed per-core inputs: angles (4,2,512) f32, coords (4,1539) f32,
    lens (4,) int.  Returns (pk, rhs_bf16): rhs is the channel-interleaved,
    j<=len masked (r_x, r_y, r_z, 1) operand, built host-side (doing it
    on-device cost ~8us of Pool time + SBUF contention with the scan)."""
    import ml_dtypes
    pk = _pk_static().copy()

    def put(nm, arr):
        o, w = COLS[nm]
        pk[:arr.shape[0], o:o + w] = arr

    # scan layout p = b*32 + ch; shifted by one position (exclusive scan input)
    ash = np.zeros((4, L), np.float32)
    bsh = np.zeros((4, L), np.float32)
    ash[:, 1:] = angles[:, 0, :-1]
    bsh[:, 1:] = angles[:, 1, :-1]
    put("a_sh", ash.reshape(128, CPOS))
    put("b_sh", bsh.reshape(128, CPOS))
    put("a_f", angles[:, 0, :].reshape(128, CPOS))

    cp = np.zeros((4, 4, NJ), np.float32)  # [b, comp, j]
    for b in range(4):
        cp[b, 0:3] = coords[b].reshape(NJ, 3).T
        cp[b, 3] = 1.0
        cp[b, :, int(lens[b]) + 1:] = 0.0
    rhs = np.zeros((48, ROW), np.float32)
    for b in range(4):
        for s_, (cch, kk) in SLOT_CK.items():
            rhs[b * 12 + s_, cch::3] = cp[b, kk]
    return pk, rhs.astype(ml_dtypes.bfloat16)


def _plan(lens):
    """Len-sorted pairing + byte-balanced core assignment.

    Returns (perm, flags): perm[4c+s] = original sample index for core c
    slot s; flags[c] = int32 (1, 16): per pair bp, flags[bp*8+n] =
    (pairmax_len > 64*n)."""
    lens = np.asarray(lens).astype(np.int64)
    order = np.argsort(lens, kind="stable")
    pairs = [(int(order[2 * m]), int(order[2 * m + 1])) for m in range(16)]

    def pair_cost(pr):
        lm = max(lens[pr[0]], lens[pr[1]])
        el = 0
        for ti in range(4):
            for k in range(2):
                if lm > 128 * ti + 64 * k:
                    el += 64 * (CW - 192 * k)
            for cj in range(ti + 1, 4):
                if lm > 128 * cj:
                    el += 128 * CW
        return el

    costs = [pair_cost(p) for p in pairs]
    core_pairs = [[] for _ in range(8)]
    core_load = [0] * 8
    for m in sorted(range(16), key=lambda i: -costs[i]):
        c = min([cc for cc in range(8) if len(core_pairs[cc]) < 2],
                key=lambda cc: core_load[cc])
        core_pairs[c].append(m)
        core_load[c] += costs[m]
    perm = np.empty(32, np.int64)
    flags = []
    for c in range(8):
        f = np.zeros((1, 16), np.int32)
        for bp, m in enumerate(core_pairs[c]):
            a, b = pairs[m]
            perm[4 * c + 2 * bp] = a
            perm[4 * c + 2 * bp + 1] = b
            lm = max(lens[a], lens[b])
            f[0, bp * 8:bp * 8 + 8] = (lm > 64 * np.arange(8)).astype(np.int32)
        flags.append(f)
    return perm, flags


def build_nc():
    import concourse.bass as bass
    import concourse.bacc as bacc
    import concourse.mybir as mybir
    from concourse.tile import TileContext

    F32 = mybir.dt.float32
    F32R = mybir.dt.float32r
    OP = mybir.AluOpType
    ACT = mybir.ActivationFunctionType

    nc = bacc.Bacc(target_bir_lowering=False, trn_type="TRN2")

    BF16 = mybir.dt.bfloat16
    pk_in = nc.declare_dram_parameter("pk", [128, PKW], F32, isOutput=False)
    rhs_in = nc.declare_dram_parameter("rhs", [48, ROW], BF16, isOutput=False)
    flg_in = nc.declare_dram_parameter("flg", [1, 16], mybir.dt.int32,
                                       isOutput=False)
    # Output in bf16 (tolerance 2e-2 >> bf16 rounding); host converts to f32.
    out = nc.declare_dram_parameter("out", [4, 2, GP], BF16, isOutput=True)
    bounce1 = nc.dram_tensor("bounce1", [24 * 2048], BF16)

    def dram_ap(handle, offset, dims):
        return bass.AP(tensor=handle, offset=offset,
                       ap=[list(d) for d in dims])

    def view(ap, offset, dims):
        """Free-dim view of an SBUF AP: keep its partition dim, custom free dims."""
        return bass.AP(tensor=ap.tensor, offset=ap.offset + offset,
                       ap=[list(ap.ap[0])] + [list(d) for d in dims])

    with TileContext(nc) as tc, tc.tile_pool(name="main", bufs=1) as MP:
        def T(shape, name):
            return MP.tile(shape, F32, name=name, tag=name)

        # ONE SBUF TILE PER DEPENDENCY STAGE: readers of a tile wait on the
        # whole tile's write set, so the trig-critical 24KB must not share a
        # tile with the 845KB of constants.  Sync kicks the scan-critical
        # splits in consumer order; Scalar kicks rhs + flg in parallel.
        _splits = [("t_trig", 0, 48),        # angles (trig, 24KB, FIRST)
                   ("t_sgn", 48, 256),       # sgncm (round 1)
                   ("t_shm", 304, 660),      # shm, efq (cross-chunk scan)
                   ("t_main", COLS["tmat"][0], 640)]   # tmat + trimask
        _ptile = {}
        for (tn, o, w) in _splits:
            t_ = T([128, w], tn)
            nc.sync.dma_start(t_[:], pk_in[:, o:o + w])
            for nm, (co, cw) in COLS.items():
                if o <= co and co + cw <= o + w:
                    _ptile[nm] = (t_, co - o)
        # One [12, ROW] tile per sample (matmul operands must share base
        # partition 0 with the lhsT tile).
        rhs = []
        for b in range(4):
            rb = MP.tile([12, ROW], BF16, name=f"rhs{b}", tag=f"rhs{b}")
            rhs.append(rb)
            nc.sync.dma_start(rb[:], rhs_in[b * 12:b * 12 + 12, :])
        flg = MP.tile([1, 16], mybir.dt.int32, name="flg_sb", tag="flg_sb")
        nc.sync.dma_start(flg[:], flg_in[0:1, :])


        def PKV(nm, rows=128):
            t_, o = _ptile[nm]
            w = COLS[nm][1]
            return t_[0:rows, o:o + w]

        # ---- trig (wrap into [-pi, pi]: Sin LUT range limit) ----
        PI = float(np.pi)
        cAs, sAs = T([128, CPOS], "cAs"), T([128, CPOS], "sAs")
        cBs, sBs = T([128, CPOS], "cBs"), T([128, CPOS], "sBs")
        caf, saf = T([128, CPOS], "caf"), T([128, CPOS], "saf")
        wt1 = T([128, CPOS], "wt1")
        wt2 = T([128, CPOS], "wt2")
        wt3 = T([128, CPOS], "wt3")
        wt4 = T([128, CPOS], "wt4")
        for src, scale, outs in (("a_sh", 0.5, (cAs, sAs)),
                                 ("b_sh", 0.5, (cBs, sBs)),
                                 ("a_f", 1.0, (caf, saf))):
            eng = nc.vector
            wta, wtb = (wt3, wt4) if scale == 1.0 else (wt1, wt2)
            for (dst, shift) in ((outs[0], PI / 2), (outs[1], 0.0)):
                y = T([128, CPOS], f"y_{src}_{int(shift * 10)}")
                eng.tensor_scalar(y[:], PKV(src), scale, shift,
                                  OP.mult, OP.add)
                if scale == 0.5 and shift == 0.0:
                    # |x/2| < pi for N(0,1) inputs: no wrap needed
                    nc.scalar.activation(dst[:], y[:], ACT.Sin, bias=0.0,
                                         scale=1.0)
                    continue
                wrapt = T([128, CPOS], f"wr_{src}_{int(shift * 10)}")
                eng.tensor_scalar(wta[:], y[:], PI, None, OP.is_gt)
                if scale == 0.5:
                    # x/2 + pi/2 can only overflow the upper bound
                    eng.scalar_tensor_tensor(wrapt[:], wta[:], -2 * PI,
                                             y[:], OP.mult, OP.add)
                else:
                    eng.tensor_scalar(wtb[:], y[:], -PI, None, OP.is_lt)
                    eng.tensor_tensor(wta[:], wta[:], wtb[:], OP.subtract)
                    eng.scalar_tensor_tensor(wrapt[:], wta[:], -2 * PI,
                                             y[:], OP.mult, OP.add)
                nc.scalar.activation(dst[:], wrapt[:], ACT.Sin, bias=0.0,
                                     scale=1.0)

        C = T([128, 24 * CPOS], "Cstack")
        # Only slots {0,5,10} (+12 for g1) stay zero (cross-product diagonal);
        # all others are written below. Strided vector memsets beat a full
        # [128, 384] gpsimd memset (~1.3us measured).
        nc.vector.memset(view(C[:], 0, [[192, 2], [1, 3 * CPOS]]), 0.0)

        def slot(s_):
            return C[:, s_ * CPOS:(s_ + 1) * CPOS]

        with tc.tile_pool(name="scan", bufs=2) as SP, \
             tc.tile_pool(name="scantmp", bufs=2) as TP, \
             tc.tile_pool(name="pscan", bufs=2, space="PSUM") as PS:
            # local quats q = (cA cB, cA sB, sA sB, sA cB), from shifted
            # angles.  CHANNEL-MAJOR state with an 8-col IDENTITY PAD per
            # channel block (free idx = c*24 + 8 + pos): round s reads its
            # shifted a-operand at offset 8-s, so pos<s lands in the pad and
            # composes with identity -- no prefix copy, and every round op
            # is full-width with contiguous innermost dims.
            PAD = 8
            BS = CPOS + PAD
            Pa = SP.tile([128, 4 * BS], F32, name="scanP0", tag="scanP0")
            Pb = SP.tile([128, 4 * BS], F32, name="scanP1", tag="scanP1")
            for Pt in (Pa, Pb):
                nc.vector.memset(view(Pt[:], 0, [[1, PAD]]), 1.0)
                nc.vector.memset(view(Pt[:], BS, [[BS, 3], [1, PAD]]), 0.0)
            for ci, (x, y) in enumerate(((cAs, cBs), (cAs, sBs), (sAs, sBs), (sAs, cBs))):
                nc.vector.tensor_tensor(
                    Pa[:, ci * BS + PAD:ci * BS + PAD + CPOS],
                    x[:], y[:], OP.mult)
            # identity quat at i=0 of each sample comes free: a_sh/b_sh are
            # zero-filled at pos 0 so q = (cos0*cos0, ...) = (1, 0, 0, 0)
            # (Sin LUT exactness at 0 / pi/2 is ~1e-5, far under tolerance).

            def quat_round_cm(a_src, a_ps, a_ks, b_src, b_off, nxt, out_off,
                              out_cs, npos, cs, eng):
                """nxt[out_off + c*out_cs + pos] =
                       sum_k sgn[c,k] * a[pos*a_ps + k*a_ks]
                                      * b[b_off + (k^c)*cs + pos].
                   Dim order (c, k, pos) everywhere: every operand has
                   stride-1 or stride-0 innermost (DVE chokes on strided
                   innermost dims).  k-sum done as two contiguous adds.
                   All on `eng` so the round has no cross-engine sync."""
                n4 = npos * 4
                sa = TP.tile([128, 256], F32, name="sa", tag="sa")
                eng.tensor_tensor(
                    view(sa[:], 0, [[n4, 4], [npos, 4], [1, npos]]),
                    view(a_src, 0, [[0, 4], [a_ks, 4], [a_ps, npos]]),
                    view(PKV("sgncm"), 0, [[64, 4], [16, 4], [1, npos]]),
                    OP.mult)
                v = TP.tile([128, 256], F32, name="vv", tag="vv")
                for c in range(4):
                    s2, s1 = _SGN_B[c]
                    eng.tensor_tensor(
                        view(v[:], c * n4, [[2 * npos, 2], [npos, 2], [1, npos]]),
                        view(sa[:], c * n4, [[2 * npos, 2], [npos, 2], [1, npos]]),
                        view(b_src, b_off + c * cs,
                             [[s2 * cs, 2], [s1 * cs, 2], [1, npos]]),
                        OP.mult)
                t2 = TP.tile([128, 128], F32, name="t2", tag="t2")
                eng.tensor_tensor(
                    view(t2[:], 0, [[2 * npos, 4], [npos, 2], [1, npos]]),
                    view(v[:], 0, [[n4, 4], [2 * npos, 2], [1, npos]]),
                    view(v[:], npos, [[n4, 4], [2 * npos, 2], [1, npos]]),
                    OP.add)
                eng.tensor_tensor(
                    view(nxt, out_off, [[out_cs, 4], [1, npos]]),
                    view(t2[:], 0, [[2 * npos, 4], [1, npos]]),
                    view(t2[:], npos, [[2 * npos, 4], [1, npos]]),
                    OP.add)

            cur_t, nxt_t = Pa, Pb
            for s in (1, 2, 4, 8):      # in-chunk shifts (free dim)
                sa = TP.tile([128, 256], F32, name="sa", tag="sa")
                nc.vector.tensor_tensor(
                    view(sa[:], 0, [[64, 4], [16, 4], [1, 16]]),
                    view(cur_t[:], PAD - s, [[0, 4], [BS, 4], [1, 16]]),
                    view(PKV("sgncm"), 0, [[64, 4], [16, 4], [1, 16]]),
                    OP.mult)
                v = TP.tile([128, 256], F32, name="vv", tag="vv")
                for c in range(4):
                    s2, s1 = _SGN_B[c]
                    nc.vector.tensor_tensor(
                        view(v[:], c * 64, [[32, 2], [16, 2], [1, 16]]),
                        view(sa[:], c * 64, [[32, 2], [16, 2], [1, 16]]),
                        view(cur_t[:], PAD + c * BS,
                             [[s2 * BS, 2], [s1 * BS, 2], [1, 16]]),
                        OP.mult)
                t2 = TP.tile([128, 128], F32, name="t2", tag="t2")
                nc.vector.tensor_tensor(
                    view(t2[:], 0, [[32, 4], [16, 2], [1, 16]]),
                    view(v[:], 0, [[64, 4], [32, 2], [1, 16]]),
                    view(v[:], 16, [[64, 4], [32, 2], [1, 16]]), OP.add)
                nc.vector.tensor_tensor(
                    view(nxt_t[:], PAD, [[BS, 4], [1, 16]]),
                    view(t2[:], 0, [[32, 4], [1, 16]]),
                    view(t2[:], 16, [[32, 4], [1, 16]]), OP.add)
                cur_t, nxt_t = nxt_t, cur_t
            # cross-chunk: Hillis-Steele over chunk totals (PE shift-matmul).
            # State in bf16: fp32 operands make every shift matmul a 2-pass
            # (2x LDWEIGHTS+MATMUL); the 0/1 shift matrix is exact in bf16
            # and 6 rounds of fp16 state rounding stay inside the 2e-2
            # budget (bf16 measured 2.8e-2: too coarse).  shm weights converted once on Scalar (off-path).
            F16 = mybir.dt.float16
            shmb = MP.tile([128, 640], F16, name="shmb", tag="shmb")
            nc.scalar.copy(shmb[:], view(PKV("shm1"), 0, [[1, 640]]))
            _shb = {d: shmb[:, i * 128:(i + 1) * 128]
                    for i, d in enumerate((1, 2, 4, 8, 16))}
            tot = SP.tile([128, 4], F16, name="tot0", tag="tot")
            nc.vector.tensor_copy(tot[:], view(cur_t[:], PAD + CPOS - 1, [[BS, 4]]))
            def cross_round(sh_ps, b_tot, ntot):
                sa = TP.tile([128, 16], F32, name="xsa", tag="xsa")
                nc.vector.tensor_tensor(
                    view(sa[:], 0, [[4, 4], [1, 4]]),
                    view(sh_ps, 0, [[0, 4], [1, 4]]),
                    view(PKV("sgncm"), 0, [[64, 4], [16, 4]]), OP.mult)
                v = TP.tile([128, 16], F32, name="xvv", tag="xvv")
                for c in range(4):
                    s2, s1 = _SGN_B[c]
                    nc.vector.tensor_tensor(
                        view(v[:], c * 4, [[2, 2], [1, 2]]),
                        view(sa[:], c * 4, [[2, 2], [1, 2]]),
                        view(b_tot, c, [[s2, 2], [s1, 2]]), OP.mult)
                with nc.allow_low_precision(
                        reason="4-elem quat k-sum to fp16 state; bounded "
                               "unit quats, rel ~5e-4/round"):
                    nc.vector.tensor_reduce(
                        view(ntot, 0, [[1, 4]]),
                        view(v[:], 0, [[4, 4], [1, 4]]),
                        mybir.AxisListType.X, OP.add)

            for d in (1, 2, 4, 8, 16):
                sh_ps = PS.tile([128, 4], F32, name=f"shps{d}", tag="shps")
                nc.tensor.matmul(sh_ps[:], _shb[d], tot[:],
                                 start=True, stop=True)
                qt = TP.tile([128, 4], F16, name=f"qt{d}", tag="qt")
                nc.vector.tensor_tensor(qt[:], sh_ps[:], PKV(f"efq{d}"), OP.add)
                ntot = SP.tile([128, 4], F16, name=f"tot{d}", tag="tot")
                cross_round(qt[:], tot[:], ntot[:])
                tot = ntot
            # exclusive chunk offsets = totscan shifted one chunk (+identity)
            off_ps = PS.tile([128, 4], F32, name="off_ps", tag="shps")
            nc.tensor.matmul(off_ps[:], _shb[1], tot[:],
                             start=True, stop=True)
            offq = SP.tile([128, 4], F32, name="offq", tag="tot")
            nc.vector.tensor_tensor(offq[:], off_ps[:], PKV("efq1"), OP.add)
            # compose: final[p, c, pos] = (offq[p] (x) cur[p, :, pos])_c
            nxt = SP.tile([128, 64], F32, name="scan_fin", tag="scan")
            quat_round_cm(offq[:], 0, 1, cur_t[:], PAD, nxt[:], 0, CPOS,
                          CPOS, BS, nc.vector)
            cur = nxt

            # ---- conversion: Qex -> w/nu planes + crosses into C ----
            # No row (i < len) masking: rows with i >= len only ever hit
            # output cols with j > i >= len, which the rhs j<=len mask
            # already zeroes.  R = I + 2*(...): the 2x is folded into the
            # products via scalar_tensor_tensor.
            W = cur[:, 0:CPOS]
            X = cur[:, CPOS:2 * CPOS]
            Y = cur[:, 2 * CPOS:3 * CPOS]
            Z = cur[:, 3 * CPOS:4 * CPOS]

            # grouped products: dbl = 2*[X|Y|Z]; then 4 wide multiplies
            # cover all nine 2*q_i*q_j products.
            dbl = T([128, 48], "dbl")
            nc.vector.tensor_scalar(dbl[:], cur[:, CPOS:4 * CPOS], 2.0,
                                    None, OP.mult)
            PG1 = T([128, 48], "PG1")   # [wx2, wy2, wz2]
            nc.vector.tensor_tensor(view(PG1[:], 0, [[16, 3], [1, CPOS]]),
                                    view(cur[:], 0, [[0, 3], [1, CPOS]]),
                                    dbl[:], OP.mult)
            PG2 = T([128, 48], "PG2")   # [xx2, yy2, zz2]
            nc.vector.tensor_tensor(PG2[:], cur[:, CPOS:4 * CPOS],
                                    dbl[:], OP.mult)
            PG3 = T([128, 32], "PG3")   # [xy2, yz2]
            nc.vector.tensor_tensor(PG3[:], dbl[:, 0:32],
                                    cur[:, 2 * CPOS:4 * CPOS], OP.mult)
            PG4 = T([128, CPOS], "PG4")  # [xz2]
            nc.vector.tensor_tensor(PG4[:], dbl[:, 0:16],
                                    cur[:, 3 * CPOS:4 * CPOS], OP.mult)
            pr = {"wx": PG1[:, 0:16], "wy": PG1[:, 16:32],
                  "wz": PG1[:, 32:48], "xx": PG2[:, 0:16],
                  "yy": PG2[:, 16:32], "zz": PG2[:, 32:48],
                  "xy": PG3[:, 0:16], "yz": PG3[:, 16:32],
                  "xz": PG4[:, 0:16]}

            # col6 = [c00 c01 c02 | c10 c11 c12] contiguous for the wide
            # nu ops below.
            col6 = T([128, 96], "col6")
            col = {f"c{r}{cc}": col6[:, (3 * r + cc) * CPOS:
                                     (3 * r + cc + 1) * CPOS]
                   for r in range(2) for cc in range(3)}
            chains = [(slot(SLOT_POS[0]), "xz", "wy", OP.add, False),
                      (slot(SLOT_POS[1]), "yz", "wx", OP.subtract, False),
                      (slot(SLOT_POS[2]), "xx", "yy", OP.add, True),
                      (col["c00"], "yy", "zz", OP.add, True),
                      (col["c01"], "xy", "wz", OP.add, False),
                      (col["c02"], "xz", "wy", OP.subtract, False),
                      (col["c10"], "xy", "wz", OP.subtract, False),
                      (col["c11"], "xx", "zz", OP.add, True),
                      (col["c12"], "yz", "wx", OP.add, False)]
            ct = [T([128, CPOS], f"ct{i}") for i in range(9)]
            for i, (dst, a1, a2, op, om) in enumerate(chains):
                nc.vector.tensor_tensor(ct[i][:] if om else dst,
                                        pr[a1], pr[a2], op)
            for i, (dst, a1, a2, op, om) in enumerate(chains):
                if om:   # diagonal entries: 1 - 2*(p+q)
                    nc.vector.tensor_scalar(dst, ct[i][:], -1.0, 1.0,
                                            OP.mult, OP.add)
            # nu = col0*cos a + col1*sin a, all 3 comps in one 48-col op
            # each; result lands in the contiguous +nu slots 12+3..12+5.
            nut0 = T([128, 48], "nut0")
            nut1 = T([128, 48], "nut1")
            nc.vector.tensor_tensor(nut0[:], col6[:, 0:48],
                                    view(caf[:], 0, [[0, 3], [1, CPOS]]),
                                    OP.mult)
            nc.vector.tensor_tensor(nut1[:], col6[:, 48:96],
                                    view(saf[:], 0, [[0, 3], [1, CPOS]]),
                                    OP.mult)
            nc.vector.tensor_tensor(C[:, (12 + 3) * CPOS:(12 + 6) * CPOS],
                                    nut0[:], nut1[:], OP.add)
            # negations: one 48-col op per plane (slots 6-8 <- 3-5)
            for g0 in (0, 12):
                nc.vector.tensor_scalar(
                    C[:, (g0 + 6) * CPOS:(g0 + 9) * CPOS],
                    C[:, (g0 + 3) * CPOS:(g0 + 6) * CPOS],
                    -1.0, None, OP.mult)

            # ---- s_ex = R_CA * exclusive-cumsum(nu) ----
            zeros16 = T([128, CPOS], "zeros16")
            nc.vector.memset(zeros16[:], 0.0)
            nu_incl = MP.tile([128, 48], F16, name="nu_incl", tag="nu_incl")
            tmat16 = MP.tile([128, 128], F16, name="tmat16", tag="tmat16")
            nc.scalar.copy(tmat16[:], PKV("tmat"))
            with nc.allow_low_precision(
                    reason="fp16 nu cumsum over 16 positions, |nu|<=1: "
                           "~1e-3 rel vs 2e-2 budget; buys single-pass "
                           "fp16 tmat matmul"):
                for cc in range(3):
                    nc.vector.tensor_tensor_scan(
                        nu_incl[:, cc * CPOS:(cc + 1) * CPOS],
                        slot(12 + SLOT_POS[cc]), zeros16[:], 0.0,
                        OP.add, OP.add)
            offs_ps = PS.tile([128, 4], F32, name="offs_ps", tag="shps")
            nc.tensor.matmul(offs_ps[:, 0:3], tmat16[:],
                             view(nu_incl[:], CPOS - 1, [[CPOS, 3]]),
                             start=True, stop=True)
            offs = T([128, 3], "offs")
            nc.vector.tensor_copy(offs[:], offs_ps[:, 0:3])
            s_ex = T([128, 48], "s_ex")
            for cc in range(3):
                nc.vector.tensor_copy(s_ex[:, cc * CPOS:cc * CPOS + 1],
                                      offs[:, cc:cc + 1])
            for cc in range(3):
                nc.vector.tensor_scalar(
                    s_ex[:, cc * CPOS + 1:(cc + 1) * CPOS],
                    nu_incl[:, cc * CPOS:(cc + 1) * CPOS - 1],
                    R_CA, offs[:, cc:cc + 1], OP.mult, OP.add)

            def sc_(cc):
                return s_ex[:, cc * CPOS:(cc + 1) * CPOS]

            crt = {(e, i): T([128, CPOS], f"crt{e}{i}")
                   for e in (0, 1) for i in range(6)}
            for base in (0, 12):  # (s x v)_c = s_{c+1} v_{c+2} - s_{c+2} v_{c+1}
                ei = 0 if base == 0 else 1
                eng = nc.gpsimd if base == 0 else nc.vector
                for cc in range(3):  # staged: products first, then subtracts
                    c1, c2 = (cc + 1) % 3, (cc + 2) % 3
                    eng.tensor_tensor(crt[ei, 2 * cc][:], sc_(c1),
                                      slot(base + SLOT_POS[c2]), OP.mult)
                    eng.tensor_tensor(crt[ei, 2 * cc + 1][:], sc_(c2),
                                      slot(base + SLOT_POS[c1]), OP.mult)
                for cc in range(3):
                    eng.tensor_tensor(slot(base + SLOT_CRS[cc]),
                                      crt[ei, 2 * cc][:],
                                      crt[ei, 2 * cc + 1][:], OP.subtract)

        # ---- C -> bf16 -> bounce1 -> lhsT (12, [g, b, i]) ----
        Cb = MP.tile([128, 24 * CPOS], BF16, name="Cb", tag="Cb")
        Cb3 = Cb.rearrange("p (slot pos) -> p slot pos", slot=24)
        lhsT = MP.tile([12, 4096], BF16, name="lhsT", tag="lhsT")
        for g in range(2):   # per-plane bounce: g=0 kicks as soon as the
            # ga slots are done, so production isn't gated on gb's crosses.
            # Each plane split into partition halves so the read-back of
            # half 0 overlaps the write-out of half 1 (cuts latency).
            nc.scalar.copy(Cb[:, g * 192:(g + 1) * 192],
                           C[:, g * 192:(g + 1) * 192])
            for h in range(2):
                nc.sync.dma_start(
                    dram_ap(bounce1, g * 12 * 2048 + h * 1024,
                            [[16, 64], [2048, 12], [1, 16]]),
                    Cb3[64 * h:64 * h + 64, g * 12:(g + 1) * 12, :])
                nc.sync.dma_start(
                    lhsT[:, g * 2048 + h * 1024:g * 2048 + h * 1024 + 1024],
                    dram_ap(bounce1, g * 12 * 2048 + h * 1024,
                            [[2048, 12], [1, 1024]]))

        # rhs (channel-interleaved, j<=len masked) is built host-side and
        # DMAed straight into rhs48; matmuls slice partition ranges b*12..
        # Below-diagonal zeros are never written: SPMD output buffers are
        # donated pre-zeroed (bass2jax.run_bass_via_pjrt zero-fills them).

        # Per-pair cond flags: pair 0 on Sync regs, pair 1 on Scalar regs
        # (the two HWDGE engines; each kicks its pair's output DMAs).
        _, cond_p0 = nc.values_load_multi_w_load_instructions(
            flg[0:1, 0:8], engines=[mybir.EngineType.SP],
            min_val=0, max_val=1, skip_runtime_bounds_check=True)
        _, cond_p1 = nc.values_load_multi_w_load_instructions(
            flg[0:1, 8:16], engines=[mybir.EngineType.Activation],
            min_val=0, max_val=1, skip_runtime_bounds_check=True)
        # Third kick stream on GpSimd (idle through the main loop): DMA
        # issue on an engine serializes behind its previous DMA's drain,
        # so splitting bytes across 3 engines raises aggregate drain rate.
        _, cond_gp = nc.values_load_multi_w_load_instructions(
            flg[0:1, 0:16], engines=[mybir.EngineType.Pool],
            min_val=0, max_val=1, skip_runtime_bounds_check=True)
        conds = (cond_p0, cond_p1)
        kick_eng = (nc.sync, nc.scalar)

        # ---- main loop: weight-reusing matmuls -> ACT evict -> GpSimd mask ----
        trimask_ap = PKV("trimask")
        with tc.tile_pool(name="pmain", bufs=8, space="PSUM") as PM, \
             tc.tile_pool(name="stg", bufs=1) as SG:
            # PE p-state warmup: dummy matmuls in the PE's idle window
            # (bounce/lhsT wait) so production starts at full clock.
            for wi in range(8):
                ptw = PM.tile([128, 512], F32, name="warm", tag="pt")
                nc.tensor.matmul(ptw[:, 0:512], rhs[0][:, 0:128],
                                 rhs[0][:, 512:1024],
                                 start=True, stop=True)
            for g in range(2):
                for ti in range(4):
                    nact = CW * (4 - ti)           # active width per sample
                    n0 = CW * ti + 3               # first active column
                    stg = SG.tile([128, 4 * nact], BF16, name=f"stg{g}{ti}",
                                  tag=f"stg{g}{ti}")
                    stg4 = stg.rearrange("p (b w) -> p b w", b=4)
                    for b in range(4):
                        lh = lhsT[:, g * 2048 + b * 512 + ti * 128:
                                  g * 2048 + b * 512 + (ti + 1) * 128]
                        cuts = list(range(0, nact, 512)) + [nact]
                        for ci, (c0, c1) in enumerate(zip(cuts[:-1], cuts[1:])):
                            pt = PM.tile([128, 512], F32, name="pt", tag="pt")
                            nc.tensor.matmul(
                                pt[:, 0:c1 - c0], lh,
                                rhs[b][:, n0 + c0:n0 + c1],
                                start=True, stop=True)
                            if ci == 0:   # masked evict (diag), on Vector
                                nc.vector.tensor_tensor(
                                    stg4[:, b, 0:c1], pt[:, 0:c1],
                                    view(trimask_ap, 0, [[1, c1]]), OP.mult)
                            elif ci == 2:  # third chunk also on Vector
                                nc.vector.tensor_copy(stg4[:, b, c0:c1],
                                                      pt[:, 0:c1 - c0])
                            else:          # middle chunk on Scalar
                                nc.scalar.copy(stg4[:, b, c0:c1],
                                               pt[:, 0:c1 - c0])
                    for bp in range(2):
                        eng, cnd = kick_eng[bp], conds[bp]
                        for k in range(2):   # 64-row diagonal groups
                            eng.dma_start(
                                dram_ap(out, (2 * bp) * 2 * GP + g * GP
                                        + (ti * 128 + 64 * k) * ROW
                                        + n0 + 192 * k,
                                        [[ROW, 64], [2 * GP, 2],
                                         [1, CW - 192 * k]]),
                                stg4[64 * k:64 * k + 64,
                                     2 * bp:2 * bp + 2, 192 * k:CW],
                                cond=cnd[2 * ti + k])
                        for cj in range(ti + 1, 4):  # 128-j tail chunks
                            # first tail on the pair's own engine, the rest
                            # on the GpSimd kick stream
                            if cj == ti + 1:
                                keng, kc = eng, cnd[2 * cj]
                            else:
                                keng, kc = nc.gpsimd, cond_gp[bp * 8 + 2 * cj]
                            keng.dma_start(
                                dram_ap(out, (2 * bp) * 2 * GP + g * GP
                                        + ti * 128 * ROW + CW * cj + 3,
                                        [[ROW, 128], [2 * GP, 2], [1, CW]]),
                                stg4[:, 2 * bp:2 * bp + 2,
                                     CW * (cj - ti):CW * (cj - ti) + CW],
                                cond=kc)
    nc.compile()
    return nc


_NC_CACHE = {}


def _get_nc():
    if "nc" not in _NC_CACHE:
        _NC_CACHE["nc"] = build_nc()
    return _NC_CACHE["nc"]


def run_spmd(input_angles, input_coords, angles_length, trace=False):
    from concourse.bass_utils import run_bass_kernel_spmd

    input_angles = np.ascontiguousarray(np.asarray(input_angles, np.float32))
    input_coords = np.ascontiguousarray(np.asarray(input_coords, np.float32))
    angles_length = np.asarray(angles_length)
    assert input_angles.shape[0] == 32

    nc = _get_nc()
    perm, flags = _plan(angles_length)
    in_maps = []
    for core in range(8):
        sl = perm[core * 4:core * 4 + 4]
        pk, rhs = build_pk(input_angles[sl], input_coords[sl],
                           angles_length[sl])
        in_maps.append({"pk": pk, "rhs": rhs, "flg": flags[core]})

    res = run_bass_kernel_spmd(nc, in_maps, core_ids=list(range(8)),
                               trace=trace)
    full = np.empty((32, 2, GP), np.float32)
    for core in range(8):
        full[perm[core * 4:core * 4 + 4]] = np.asarray(
            res.results[core]["out"]).astype(np.float32).reshape(4, 2, GP)
    return full, res


def kernel(input_angles, input_coords, angles_length):
    full, _ = run_spmd(input_angles, input_coords, angles_length, trace=False)
    return full


if __name__ == "__main__":
    print("kernel module OK")

